# revision 20
# baseline (speedup 1.0000x reference)
"""Multi-head attention (B=2, T=2048, D=1024, H=16) on 8 NeuronCores.

Tensor-parallel over heads: 2 heads per core. Each core computes its
heads' QKV projection, causal attention, and a partial output
projection (its 128 columns of the concat dim); partials are summed on
the host.

Device dataflow is fully "transposed" (feature-major):
  - host supplies x^T [D, B*T]
  - qkv^T = W_slice @ x^T        (per-core W rows, pre-transposed host-side)
  - S^T[k,q] block = matmul(lhsT=K^T tile, rhs=Q^T tile), contraction dh=64
  - P^T = exp(S^T/8) * causal_mask (mask only on diagonal-band blocks)
  - O_aug^T [65, q] = V_aug.T @ P^T  with V_aug = [V | 1] so row 64
    accumulates the softmax denominator for free
  - normalize with reciprocal + PE outer-product broadcast
  - y^T partial [D, B*T] = (W_out slice)^T.T @ concatO^T

All matmuls run in float32r (full-rate fp32). Scheduling notes:
  - activations live in per-512-token tiles, V_aug per (b,h,kblock),
    so the Tile scheduler overlaps projection/attention/out-proj
  - ScalarE runs only Exp in phase B (no act-table thrash); exp is
    batched over S-block pairs [128,1024] to amortize PSUM latency
  - DMA triggers issue from otherwise-idle engine queues (ACT for x
    loads, DVE for y stores) because a HWDGE trigger costs ~1.2us of
    queue time and SP would otherwise serialize phase A
"""

import sys

sys.path.insert(0, "/opt/trn_rl_repo")

import numpy as np

import concourse.bass as bass
import concourse.mybir as mybir
import concourse.tile as tile
from concourse import bacc
from concourse.masks import make_identity

B = 2
T = 2048
D = 1024
H = 16
DH = 64
N_CORES = 8
HPC = H // N_CORES          # heads per core = 2
F = HPC * DH                # per-core feature block = 128
TOK = B * T                 # 4096
P = 128                     # partitions
QB = 512                    # q block (free dim of S^T tiles)
KB = 128                    # k block (partition dim of S^T tiles)
NQB = T // QB               # 4 q blocks per instance
NKB = T // KB               # 16 k blocks per instance
NTT = TOK // QB             # 8 token tiles for projections
NKT = D // P                # 8 contraction tiles over D

F32 = mybir.dt.float32
F32R = mybir.dt.float32r
F16 = mybir.dt.float16
EXP = mybir.ActivationFunctionType.Exp


def build_nc(loop_n: int = 1):
    """loop_n > 1 wraps the whole kernel in an on-device For_i loop —
    used only by the timing harness to amortize dispatch overhead."""
    nc = bacc.Bacc()

    xT = nc.dram_tensor("xT", [D, TOK], F16, kind="ExternalInput")
    wqkvT = nc.dram_tensor("wqkvT", [D, 3 * F], F16, kind="ExternalInput")
    woT = nc.dram_tensor("woT", [F, D], F32R, kind="ExternalInput")
    yT = nc.dram_tensor("yT", [D, TOK], F16, kind="ExternalOutput")

    with tile.TileContext(nc) as tc:
        with (
            tc.tile_pool(name="const", bufs=1) as const,
            tc.tile_pool(name="big", bufs=1) as big,
            tc.tile_pool(name="xin", bufs=8) as xin,
            tc.tile_pool(name="psb", bufs=6) as psb,
            tc.tile_pool(name="small", bufs=2) as small,
            tc.tile_pool(name="ysb", bufs=2) as ysb,
            tc.tile_pool(name="ps2", bufs=2, space="PSUM") as ps2,
            tc.tile_pool(name="ps_o", bufs=2, space="PSUM") as ps_o,
            tc.tile_pool(name="ps_r", bufs=1, space="PSUM") as ps_r,
            tc.tile_pool(name="ps_tr", bufs=1, space="PSUM") as ps_tr,
        ):
            import contextlib

            loop_ctx = (
                tc.For_i(0, loop_n, 1) if loop_n > 1 else contextlib.nullcontext()
            )
            with loop_ctx:
                build_body(nc, tc, const, big, xin, psb, small, ysb,
                           ps2, ps_o, ps_r, ps_tr, xT, wqkvT, woT, yT)

    nc.compile()
    return nc


def build_body(nc, tc, const, big, xin, psb, small, ysb,
               ps2, ps_o, ps_r, ps_tr, xT, wqkvT, woT, yT):
    if True:
        if True:
            # ---- constants ----
            ident = const.tile([P, P], F32, tag="ident")
            make_identity(nc, ident[:])
            ones32 = const.tile([P, DH], F32, tag="ones32")
            nc.gpsimd.memset(ones32[:], 1.0)
            ones_row = const.tile([1, DH], F32R, tag="ones")
            nc.vector.tensor_copy(ones_row[:], ones32[0:1, :])
            ones_col = const.tile([P, 1], F16, tag="ones_col")
            nc.vector.tensor_copy(ones_col[:], ones32[:, 0:1])
            # masks[j][krow, qcol] = 1.0 if qcol - krow - 128*j >= 0 else 0
            masks = []
            for j in range(QB // KB):
                m32 = const.tile([P, QB], F32, tag="mask32", name=f"mask32_{j}")
                nc.gpsimd.memset(m32[:], 1.0)
                nc.gpsimd.affine_select(
                    out=m32[:],
                    in_=m32[:],
                    compare_op=mybir.AluOpType.is_ge,
                    fill=0.0,
                    base=-KB * j,
                    channel_multiplier=-1,
                    pattern=[[1, QB]],
                )
                m = const.tile([P, QB], F16, tag=f"mask_{j}", name=f"mask_{j}")
                nc.vector.tensor_copy(m[:], m32[:])
                masks.append(m)

            # ---- weights to SBUF ----
            w_sb = const.tile([P, NKT, 3 * F], F16, tag="w_sb")
            for kt in range(NKT):
                nc.sync.dma_start(
                    out=w_sb[:, kt, :],
                    in_=wqkvT[kt * P : (kt + 1) * P, :],
                )
            wo_sb = const.tile([P, D], F32R, tag="wo_sb")
            nc.sync.dma_start(out=wo_sb[:], in_=woT[:, :])

            # ---- per-512-token activation tiles (fine-grained deps) ----
            QTs = [big.tile([P, QB], F32R, tag=f"QT{i}", name=f"QTs{i}") for i in range(NTT)]
            KTs = [big.tile([P, QB], F32R, tag=f"KT{i}", name=f"KTs{i}") for i in range(NTT)]
            VTs = [big.tile([P, QB], F32, tag=f"VT{i}", name=f"VTs{i}") for i in range(NTT)]
            COs = [big.tile([P, QB], F32R, tag=f"CO{i}", name=f"COs{i}") for i in range(NTT)]
            # V_aug tiles: per (b, h, ki): [128 tok, 65] (col 64 = 1.0)
            Vaugs = [
                big.tile([P, DH + 1], F16, tag=f"Va{vi}", name=f"Vaug{vi}")
                for vi in range(B * HPC * NKB)
            ]

            # ---- phase A: qkv^T = W @ x^T  (two token tiles per load) ----
            for tp in range(NTT // 2):
                xts = [
                    xin.tile([P, 2 * QB], F16, name=f"xt_{tp}_{kt}", tag="xt")
                    for kt in range(NKT)
                ]
                for kt in range(NKT):
                    # issue via SWDGE from the idle Pool queue (SP/ACT busy)
                    nc.gpsimd.dma_start(
                        out=xts[kt][:],
                        in_=xT[kt * P : (kt + 1) * P, tp * 2 * QB : (tp + 1) * 2 * QB],
                    )
                for half in range(2):
                    tt = tp * 2 + half
                    pr01 = ps2.tile([P, 2 * QB], F32, tag="s2", name=f"pr01_{tt}")
                    pr2 = ps_o.tile([P, QB], F32, tag="o", name=f"pr2_{tt}")
                    for kt in range(NKT):
                        st, sp = (kt == 0), (kt == NKT - 1)
                        xsl = xts[kt][:, half * QB : (half + 1) * QB]
                        nc.tensor.matmul(
                            pr01[:, 0:QB], w_sb[:, kt, 0:F], xsl, start=st, stop=sp
                        )
                        nc.tensor.matmul(
                            pr01[:, QB:], w_sb[:, kt, F : 2 * F], xsl, start=st, stop=sp
                        )
                        nc.tensor.matmul(
                            pr2[:], w_sb[:, kt, 2 * F :], xsl, start=st, stop=sp
                        )
                    nc.vector.tensor_copy(QTs[tt][:], pr01[:, 0:QB])
                    nc.vector.tensor_copy(KTs[tt][:], pr01[:, QB:])
                    nc.vector.tensor_copy(VTs[tt][:], pr2[:])

            # ---- phase A2: V^T -> V_aug (token-major) via PE transpose ----
            for b in range(B):
                for h in range(HPC):
                    hsl = np.s_[h * DH : (h + 1) * DH]
                    for ki in range(NKB):
                        src = VTs[b * NQB + ki // 4][
                            hsl, (ki % 4) * KB : (ki % 4 + 1) * KB
                        ]
                        tr = ps_tr.tile([P, DH], F32, tag="tr")
                        nc.tensor.matmul(
                            tr[:], src, ident[hsl, hsl],
                            is_transpose=True, start=True, stop=True,
                        )
                        va = Vaugs[(b * HPC + h) * NKB + ki]
                        nc.vector.tensor_copy(va[:, 0:DH], tr[:])
                        nc.vector.tensor_copy(va[:, DH : DH + 1], ones_col[:])

            # ---- phase B: attention per (b, h) ----
            for b in range(B):
                for h in range(HPC):
                    qrows = np.s_[h * DH : (h + 1) * DH]
                    for qi in range(NQB):
                        qt = QTs[b * NQB + qi]
                        o_ps = ps_o.tile([DH + 1, QB], F32, tag="o")
                        nblk = (qi + 1) * (QB // KB)
                        ndiag = QB // KB
                        # non-diagonal blocks: full width, exp batched in pairs
                        for k2 in range((nblk - ndiag) // 2):
                            s2 = ps2.tile([P, 2 * QB], F32, tag="s2")
                            p2 = psb.tile([P, 2 * QB], F16, tag="p")
                            for half in range(2):
                                ki = 2 * k2 + half
                                kt_tile = KTs[b * NQB + ki // 4]
                                nc.tensor.matmul(
                                    s2[:, half * QB : (half + 1) * QB],
                                    kt_tile[qrows, (ki % 4) * KB : (ki % 4 + 1) * KB],
                                    qt[qrows, :],
                                    start=True, stop=True,
                                )
                            nc.scalar.activation(p2[:], s2[:], EXP, scale=0.125)
                            for half in range(2):
                                ki = 2 * k2 + half
                                nc.tensor.matmul(
                                    o_ps[:],
                                    Vaugs[(b * HPC + h) * NKB + ki][:],
                                    p2[:, half * QB : (half + 1) * QB],
                                    start=(ki == 0),
                                    stop=False,
                                )
                        # diagonal band: columns < 128*j are fully masked; slice
                        for j in range(ndiag):
                            ki = nblk - ndiag + j
                            c0 = j * KB
                            kt_tile = KTs[b * NQB + ki // 4]
                            s2 = ps2.tile([P, 2 * QB], F32, tag="s2")
                            p2 = psb.tile([P, 2 * QB], F16, tag="p")
                            nc.tensor.matmul(
                                s2[:, 0 : QB - c0],
                                kt_tile[qrows, (ki % 4) * KB : (ki % 4 + 1) * KB],
                                qt[qrows, c0:QB],
                                start=True, stop=True,
                            )
                            nc.scalar.activation(
                                p2[:, 0 : QB - c0], s2[:, 0 : QB - c0], EXP,
                                scale=0.125,
                            )
                            nc.vector.tensor_mul(
                                p2[:, 0 : QB - c0],
                                p2[:, 0 : QB - c0],
                                masks[j][:, c0:QB],
                            )
                            nc.tensor.matmul(
                                o_ps[:, c0:QB],
                                Vaugs[(b * HPC + h) * NKB + ki][:],
                                p2[:, 0 : QB - c0],
                                start=(qi == 0 and j == 0),
                                stop=(j == ndiag - 1),
                            )
                        # normalize: rows 0:64 / row 64
                        d_sb = small.tile([1, QB], F32, tag="d")
                        nc.vector.tensor_copy(d_sb[:], o_ps[DH : DH + 1, :])
                        r_sb = small.tile([1, QB], F32R, tag="rcp")
                        with nc.allow_low_precision(reason="softmax recip bcast"):
                            nc.vector.reciprocal(r_sb[:], d_sb[:])
                        r_ps = ps_r.tile([DH, QB], F32, tag="r")
                        nc.tensor.matmul(
                            r_ps[:], ones_row[:], r_sb[:],
                            start=True, stop=True,
                        )
                        rr_sb = psb.tile([DH, QB], F32R, tag="rr", bufs=2)
                        nc.vector.tensor_copy(rr_sb[:], r_ps[:])
                        nc.vector.tensor_mul(
                            COs[b * NQB + qi][qrows, :], o_ps[0:DH, :], rr_sb[:]
                        )

            # ---- phase C: y^T partial = woT.T @ CO ----
            for b in range(B):
                for oi in range(D // P):
                    y_sb = ysb.tile([P, T], F16, name=f"ysb_{b}_{oi}", tag="y")
                    for q2 in range(NQB // 2):
                        y2 = ps2.tile([P, 2 * QB], F32, tag="s2")
                        for half in range(2):
                            qb = 2 * q2 + half
                            nc.tensor.matmul(
                                y2[:, half * QB : (half + 1) * QB],
                                wo_sb[:, oi * P : (oi + 1) * P],
                                COs[b * NQB + qb][:],
                                start=True, stop=True,
                            )
                        if (oi + q2) % 2 == 0:
                            nc.scalar.copy(
                                y_sb[:, q2 * 2 * QB : (q2 + 1) * 2 * QB], y2[:]
                            )
                        else:
                            nc.vector.tensor_copy(
                                y_sb[:, q2 * 2 * QB : (q2 + 1) * 2 * QB], y2[:]
                            )
                    # issue store from SP queue (idle after phase A)
                    nc.sync.dma_start(
                        out=yT[oi * P : (oi + 1) * P, b * T : (b + 1) * T],
                        in_=y_sb[:],
                    )


_NC = None


def get_nc():
    global _NC
    if _NC is None:
        _NC = build_nc()
    return _NC


def make_core_inputs(x, W_in, W_out):
    """Host-side sharding: per-core input maps."""
    xTh = np.ascontiguousarray(x.reshape(TOK, D).T).astype(np.float16)
    in_maps = []
    for c in range(N_CORES):
        rows = np.concatenate(
            [W_in[i * D + c * F : i * D + (c + 1) * F] for i in range(3)], axis=0
        )  # [384, 1024] = q|k|v rows for this core's 2 heads
        wqkvTh = np.ascontiguousarray(rows.T).astype(np.float16)
        woTh = np.ascontiguousarray(W_out[:, c * F : (c + 1) * F].T).astype(
            np.float32
        )
        in_maps.append({"xT": xTh, "wqkvT": wqkvTh, "woT": woTh})
    return in_maps


def kernel(x, W_in, W_out):
    from concourse.bass_utils import run_bass_kernel_spmd

    nc = get_nc()
    in_maps = make_core_inputs(
        np.asarray(x, dtype=np.float32),
        np.asarray(W_in, dtype=np.float32),
        np.asarray(W_out, dtype=np.float32),
    )
    res = run_bass_kernel_spmd(nc, in_maps, list(range(N_CORES)))
    y = np.zeros((D, TOK), dtype=np.float32)
    for r in res.results:
        y += r["yT"].astype(np.float32)
    return np.ascontiguousarray(y.T).reshape(B, T, D)
